# revision 2
# baseline (speedup 1.0000x reference)
"""Trainium2 Bass kernel for nn_BDH_GPU (sparse linear-attention decoder).

Self-contained: builds an SPMD Bass/Tile program for 8 NeuronCores,
shards batch(2) x head-groups(4), runs via PJRT (axon), gathers output.

Sharding: core c -> batch b=c//4, heads [4*(c%4), 4*(c%4)+4).
Per-layer AllReduce (split into two T-halves) within each 4-core group.

v2 restructure vs baseline:
- t-chunks of 512; all activation/state tiles split by T-half so next
  layer's x-matmuls can overlap this layer's tail (half pipelining).
- LN(a) scale folded past the z-matmul and the relu:
  y = relu(z_raw) * x * rs, so z-matmuls start straight from the a
  evacuation; the LN stats run in parallel off the critical path.
- reciprocal_approx_fast instead of vector.reciprocal (~5x).
- GpSimd only does residual adds + collective trigger (its sem ops are
  ~1.4us each); rope/sq/evacs balanced across DVE+ACT.
- AllReduce split in two T-halves: AR(half 1) overlaps tail(half 0).
"""
import numpy as np
import ml_dtypes

import concourse.bass as bass
import concourse.tile as tile
import concourse.mybir as mybir
from concourse import bacc, bass2jax

AF = mybir.ActivationFunctionType
FP32 = mybir.dt.float32
BF16 = mybir.dt.bfloat16
ts = bass.ts

D, H, N, VOCAB, L, SD, B, T = 1024, 16, 8192, 32000, 4, 512, 2, 1024
NCORES = 8
NHC = 4           # heads per core
VSH = VOCAB // 4  # vocab shard per core (within batch group) = 8000
VCH = 500         # vocab N-chunk (<=512 f32 psum bank)
NVC = VSH // VCH  # 16
EPS = 1e-5

_CACHE = {}


def build_program(nlayers=L, repeat=1, do_readout=True, collective=True):
    nc = bacc.Bacc("TRN2", target_bir_lowering=False, debug=False,
                   num_devices=NCORES)
    CDT = BF16

    v0t_d = nc.dram_tensor("v0t", [D, T], CDT, kind="ExternalInput")
    v0n_d = nc.dram_tensor("v0n", [8, 128, 8, 128], CDT, kind="ExternalInput")
    wx_d = nc.dram_tensor("wx", [NHC, D, SD], CDT, kind="ExternalInput")
    wy_d = nc.dram_tensor("wy", [NHC, D, SD], CDT, kind="ExternalInput")
    enc_d = nc.dram_tensor("enc", [NHC * SD, D], CDT, kind="ExternalInput")
    ro_d = nc.dram_tensor("ro", [D, VSH], CDT, kind="ExternalInput")
    cos_d = nc.dram_tensor("cos", [SD // 2, T], CDT, kind="ExternalInput")
    sin_d = nc.dram_tensor("sin", [SD // 2, T], CDT, kind="ExternalInput")
    msk_d = nc.dram_tensor("msk", [4, 128, 512], CDT, kind="ExternalInput")
    out_d = nc.dram_tensor("logits", [T, VSH], FP32, kind="ExternalOutput")

    with tile.TileContext(nc) as tc:
        with (
            tc.tile_pool(name="res", bufs=1) as res,
            tc.tile_pool(name="act", bufs=1) as act,
            tc.tile_pool(name="wst", bufs=10) as wst,
            tc.tile_pool(name="sml", bufs=2) as sml,
            tc.tile_pool(name="stg", bufs=2) as stg,
            tc.tile_pool(name="psp", bufs=2, space="PSUM") as psp,
            tc.tile_pool(name="dram", bufs=2, space="DRAM") as dram,
        ):
            def PPROJ(nm):
                return psp.tile([128, 512], FP32, tag="proj", bufs=2, name=nm)

            def PSC(nm):
                return psp.tile([128, 512], FP32, tag="scp", bufs=2, name=nm)

            def PAV(nm):
                return psp.tile([128, 512], FP32, tag="ap", bufs=2, name=nm)

            def PST(nm):
                return psp.tile([128, 512], FP32, tag="stp", bufs=1, name=nm)

            def PEN(nm):
                return psp.tile([128, 512], FP32, tag="encp", bufs=1, name=nm)

            # ---- constants ----
            cosv, sinv, masks = [], [], []
            for i in range(2):
                ct = res.tile([128, T], CDT, name=f"cos{i}")
                nc.sync.dma_start(ct[:], cos_d[ts(i, 128), :])
                cosv.append(ct)
                st_ = res.tile([128, T], CDT, name=f"sin{i}")
                nc.sync.dma_start(st_[:], sin_d[ts(i, 128), :])
                sinv.append(st_)
            for q in range(4):
                mt = res.tile([128, 512], CDT, name=f"msk{q}")
                nc.sync.dma_start(mt[:], msk_d[q])
                masks.append(mt)
            ones = res.tile([128, 128], CDT, name="ones")
            nc.vector.memset(ones[:], 1.0)
            epst = res.tile([128, 1], FP32, name="epst")
            nc.vector.memset(epst[:], EPS)

            # ---- persistent v state: per-(chunk, T-half) tiles ----
            vT = [[res.tile([128, 512], CDT, tag=f"vt{k}_{jj}",
                            name=f"vt{k}_{jj}") for jj in range(2)]
                  for k in range(8)]
            vn = [[res.tile([128, 4, 128], CDT, tag=f"vn{a}_{hf}",
                            name=f"vn{a}_{hf}") for hf in range(2)]
                  for a in range(8)]

            def load_v(sfx):
                for k in range(8):
                    for jj in range(2):
                        nc.sync.dma_start(vT[k][jj][:],
                                          v0t_d[ts(k, 128), ts(jj, 512)])
                for a in range(8):
                    for hf in range(2):
                        nc.sync.dma_start(vn[a][hf][:],
                                          v0n_d[a, :, 4 * hf:4 * hf + 4, :])

            load_v("init")

            for rep in range(repeat):
                if rep > 0:
                    load_v(f"r{rep}")

                for layer in range(nlayers):
                    tg = f"r{rep}l{layer}"
                    ytiles = {}
                    for h in range(NHC):
                        hg = f"{tg}h{h}"
                        # ---- x'T = relu(Wx_h^T @ vT) ----
                        wxt = []
                        for k in range(8):
                            w = wst.tile([128, SD], CDT, tag="wtile",
                                         name=f"wx_{hg}k{k}")
                            nc.sync.dma_start(w[:], wx_d[h, ts(k, 128), :])
                            wxt.append(w)
                        wyt = []
                        for k in range(8):
                            w = wst.tile([128, SD], CDT, tag="wytile", bufs=9,
                                         name=f"wy_{hg}k{k}")
                            nc.sync.dma_start(w[:], wy_d[h, ts(k, 128), :])
                            wyt.append(w)
                        xp = [[act.tile([128, 512], CDT, tag=f"xp{m}_{jj}",
                                        bufs=2, name=f"xp{m}_{jj}_{hg}")
                               for jj in range(2)] for m in range(4)]
                        for m in (0, 2, 1, 3):
                            for jj in range(2):
                                ps = PPROJ(f"xps_{hg}m{m}j{jj}")
                                for k in range(8):
                                    nc.tensor.matmul(
                                        ps[:], wxt[k][:, ts(m, 128)],
                                        vT[k][jj][:],
                                        start=(k == 0), stop=(k == 7))
                                nc.scalar.activation(out=xp[m][jj][:],
                                                     in_=ps[:], func=AF.Relu)
                        # ---- rope (per T-half) -> qr[i][jj] ----
                        qr = [[act.tile([128, 512], CDT, tag=f"qr{i}_{jj}",
                                        bufs=1, name=f"qr{i}_{jj}_{hg}")
                               for jj in range(2)] for i in range(4)]
                        for jj in range(2):
                            tj = ts(jj, 512)
                            for i in range(2):
                                t1 = sml.tile([128, 512], CDT, tag="rt",
                                              bufs=3, name=f"t1_{hg}i{i}j{jj}")
                                t2 = sml.tile([128, 512], CDT, tag="rt",
                                              bufs=3, name=f"t2_{hg}i{i}j{jj}")
                                nc.vector.tensor_mul(t1[:], xp[i][jj][:],
                                                     cosv[i][:, tj])
                                nc.vector.tensor_mul(t2[:], xp[2 + i][jj][:],
                                                     sinv[i][:, tj])
                                nc.vector.tensor_sub(qr[i][jj][:], t1[:], t2[:])
                                t3 = sml.tile([128, 512], CDT, tag="rt",
                                              bufs=3, name=f"t3_{hg}i{i}j{jj}")
                                t4 = sml.tile([128, 512], CDT, tag="rt",
                                              bufs=3, name=f"t4_{hg}i{i}j{jj}")
                                nc.vector.tensor_mul(t3[:], xp[i][jj][:],
                                                     sinv[i][:, tj])
                                nc.vector.tensor_mul(t4[:], xp[2 + i][jj][:],
                                                     cosv[i][:, tj])
                                nc.vector.tensor_add(qr[2 + i][jj][:],
                                                     t3[:], t4[:])

                        # ---- scores (strict lower-tri), t-chunks of 512 ----
                        sct = {}
                        for i in range(8):
                            for j2 in range(i // 4, 2):
                                ps = PSC(f"scp_{hg}i{i}j{j2}")
                                for k in range(4):
                                    nc.tensor.matmul(
                                        ps[:], qr[k][i // 4][:, ts(i % 4, 128)],
                                        qr[k][j2][:],
                                        start=(k == 0), stop=(k == 3))
                                sc = sml.tile([128, 512], CDT, tag="sc",
                                              bufs=13, name=f"sc_{hg}i{i}j{j2}")
                                if i // 4 == j2:
                                    nc.vector.tensor_mul(sc[:], ps[:],
                                                         masks[i % 4][:])
                                else:
                                    nc.scalar.activation(out=sc[:], in_=ps[:],
                                                         func=AF.Copy)
                                sct[(i, j2)] = sc

                        # per T-half: attnV + stats, then z and y
                        yt = [[act.tile([128, 512], CDT, tag=f"y{h}_{m}_{jj}",
                                        name=f"y{h}_{m}_{jj}_{tg}")
                               for jj in range(2)] for m in range(4)]
                        for j2 in range(2):
                            ns = 4 * (j2 + 1)
                            af = [act.tile([128, 512], CDT, tag="af", bufs=10,
                                           name=f"af{d8}_{j2}_{hg}")
                                  for d8 in range(8)]
                            stp = PST(f"stp_{hg}j{j2}")
                            for d8 in range(8):
                                ps = PAV(f"ap_{hg}j{j2}d{d8}")
                                for i in range(ns):
                                    nc.tensor.matmul(
                                        ps[:], vn[d8][i // 4][:, i % 4, :],
                                        sct[(i, j2)][:],
                                        start=(i == 0), stop=(i == ns - 1))
                                nc.scalar.activation(out=af[d8][:],
                                                     in_=ps[:], func=AF.Copy)
                                sq = sml.tile([128, 512], CDT, tag="sq",
                                              bufs=2, name=f"sq_{hg}j{j2}d{d8}")
                                nc.vector.tensor_mul(sq[:], af[d8][:],
                                                     af[d8][:])
                                nc.tensor.matmul(stp[:], ones[:], sq[:],
                                                 start=(d8 == 0),
                                                 stop=(d8 == 7))
                            sd = sml.tile([128, 512], FP32, tag="sd", bufs=1,
                                          name=f"sd_{hg}j{j2}")
                            nc.scalar.activation(out=sd[:], in_=stp[:],
                                                 func=AF.Sqrt, bias=epst[:],
                                                 scale=1.0 / D)
                            rs = sml.tile([128, 512], FP32, tag=f"rs{j2}",
                                          bufs=1, name=f"rs_{hg}j{j2}")
                            nc.vector.reciprocal_approx_fast(rs[:], sd[:])
                            # ---- z = Wy^T @ a ; y = relu(z)*x*rs ----
                            for m in range(4):
                                ps = PPROJ(f"zps_{hg}m{m}j{j2}")
                                for k in range(8):
                                    nc.tensor.matmul(
                                        ps[:], wyt[k][:, ts(m, 128)],
                                        af[k][:],
                                        start=(k == 0), stop=(k == 7))
                                rl = sml.tile([128, 512], CDT, tag="rl",
                                              bufs=2, name=f"rl_{hg}m{m}j{j2}")
                                nc.scalar.activation(out=rl[:], in_=ps[:],
                                                     func=AF.Relu)
                                nc.vector.tensor_mul(rl[:], rl[:],
                                                     xp[m][j2][:])
                                nc.vector.tensor_mul(yt[m][j2][:], rl[:],
                                                     rs[:])
                        ytiles[h] = yt

                    # ---- w partial = sum_h enc_h^T @ y_h (per T-half) ----
                    ar_in, ar_out = [], []
                    for jj in range(2):
                        ai = dram.tile([D, 512], CDT, tag=f"ar_in{jj}",
                                       name=f"ari{jj}_{tg}")
                        ao = dram.tile([D, 512], CDT, tag=f"ar_out{jj}",
                                       name=f"aro{jj}_{tg}")
                        ar_in.append(ai)
                        ar_out.append(ao)
                    for jj in range(2):
                        for dq in range(4):
                            ech = []
                            for kk in range(16):
                                e = wst.tile([128, 256], CDT, tag="etile",
                                             bufs=18,
                                             name=f"e_{tg}j{jj}q{dq}k{kk}")
                                nc.sync.dma_start(
                                    e[:], enc_d[ts(kk, 128), ts(dq, 256)])
                                ech.append(e)
                            for dl in range(2):
                                d8 = 2 * dq + dl
                                ps = PEN(f"ep_{tg}j{jj}d{d8}")
                                for kk in range(16):
                                    hh, mm = kk // 4, kk % 4
                                    nc.tensor.matmul(
                                        ps[:], ech[kk][:, ts(dl, 128)],
                                        ytiles[hh][mm][jj][:],
                                        start=(kk == 0), stop=(kk == 15))
                                so = stg.tile([128, 512], CDT, tag="so",
                                              bufs=2, name=f"so_{tg}j{jj}d{d8}")
                                nc.vector.tensor_copy(so[:], ps[:])
                                nc.sync.dma_start(ar_in[jj][ts(d8, 128), :],
                                                  so[:])
                        if collective:
                            nc.gpsimd.collective_compute(
                                "AllReduce", mybir.AluOpType.add,
                                replica_groups=[[0, 1, 2, 3], [4, 5, 6, 7]],
                                ins=[ar_in[jj].opt()], outs=[ar_out[jj].opt()])
                    w_srcs = ar_out if collective else ar_in

                    # ---- tail: u=LN(w); s=v+u; vnew=s*rsqrt(msq(s)+eps) ----
                    for jj in range(2):
                        wb = []
                        mwp = PEN(f"mwp_{tg}j{jj}")
                        msp = PST(f"msp_{tg}j{jj}")
                        for k in range(8):
                            wbk = sml.tile([128, 512], CDT, tag=f"wb{k}",
                                           bufs=1, name=f"wb_{tg}j{jj}k{k}")
                            nc.sync.dma_start(wbk[:],
                                              w_srcs[jj][ts(k, 128), :])
                            wb.append(wbk)
                            sqw = sml.tile([128, 512], CDT, tag="sqw", bufs=2,
                                           name=f"sqw_{tg}j{jj}k{k}")
                            nc.scalar.activation(out=sqw[:], in_=wbk[:],
                                                 func=AF.Square)
                            nc.tensor.matmul(mwp[:], ones[:], wbk[:],
                                             start=(k == 0), stop=(k == 7))
                            nc.tensor.matmul(msp[:], ones[:], sqw[:],
                                             start=(k == 0), stop=(k == 7))
                        mwn = sml.tile([128, 512], FP32, tag="tf32", bufs=4,
                                       name=f"mwn_{tg}j{jj}")
                        nc.scalar.activation(out=mwn[:], in_=mwp[:],
                                             func=AF.Copy, scale=1.0 / D)
                        vt = sml.tile([128, 512], FP32, tag="tf32", bufs=4,
                                      name=f"vt_{tg}j{jj}")
                        nc.scalar.activation(out=vt[:], in_=msp[:],
                                             func=AF.Copy, scale=1.0 / D)
                        m2 = sml.tile([128, 512], FP32, tag="tf32", bufs=4,
                                      name=f"m2_{tg}j{jj}")
                        nc.vector.tensor_mul(m2[:], mwn[:], mwn[:])
                        nc.vector.tensor_sub(vt[:], vt[:], m2[:])
                        nc.scalar.activation(out=vt[:], in_=vt[:],
                                             func=AF.Sqrt, bias=epst[:],
                                             scale=1.0)
                        rsw = sml.tile([128, 512], FP32, tag="tf32", bufs=4,
                                       name=f"rsw_{tg}j{jj}")
                        nc.vector.reciprocal_approx_fast(rsw[:], vt[:])
                        st = [sml.tile([128, 512], FP32, tag=f"st{k}", bufs=1,
                                       name=f"st{k}_{tg}j{jj}")
                              for k in range(8)]
                        ssp = PEN(f"ssp_{tg}j{jj}")
                        for k in range(8):
                            u = sml.tile([128, 512], CDT, tag="tu", bufs=2,
                                         name=f"u_{tg}j{jj}k{k}")
                            nc.vector.tensor_sub(u[:], wb[k][:], mwn[:])
                            nc.vector.tensor_mul(u[:], u[:], rsw[:])
                            nc.gpsimd.tensor_add(st[k][:], vT[k][jj][:], u[:])
                            sq2 = sml.tile([128, 512], CDT, tag="sqw", bufs=2,
                                           name=f"sq2_{tg}j{jj}k{k}")
                            nc.scalar.activation(out=sq2[:], in_=st[k][:],
                                                 func=AF.Square)
                            nc.tensor.matmul(ssp[:], ones[:], sq2[:],
                                             start=(k == 0), stop=(k == 7))
                        sd3 = sml.tile([128, 512], FP32, tag="tf32", bufs=4,
                                       name=f"sd3_{tg}j{jj}")
                        nc.scalar.activation(out=sd3[:], in_=ssp[:],
                                             func=AF.Sqrt, bias=epst[:],
                                             scale=1.0 / D)
                        rs2 = sml.tile([128, 512], FP32, tag="tf32", bufs=4,
                                       name=f"rs2_{tg}j{jj}")
                        nc.vector.reciprocal_approx_fast(rs2[:], sd3[:])
                        for k in range(8):
                            nc.vector.tensor_mul(vT[k][jj][:], st[k][:],
                                                 rs2[:])
                        # transpose vnew half -> natural layout (bf16 xbar)
                        for a in range(8):
                            for kq in range(4):
                                nc.sync.dma_start_transpose(
                                    vn[a][jj][:, kq, :],
                                    vT[a][jj][:, ts(kq, 128)])

            # ---- readout: logits = v^T @ readout_shard ----
            if do_readout:
                for nn_ in range(NVC):
                    rot = []
                    for k in range(8):
                        rtile = wst.tile([128, VCH], CDT, tag="wtile", bufs=10,
                                         name=f"ro_n{nn_}k{k}")
                        nc.sync.dma_start(
                            rtile[:], ro_d[ts(k, 128), ts(nn_, VCH)])
                        rot.append(rtile)
                    for m in range(8):
                        ps = PPROJ(f"rps_n{nn_}m{m}")
                        for k in range(8):
                            nc.tensor.matmul(ps[:, 0:VCH],
                                             vT[k][m // 4][:, ts(m % 4, 128)],
                                             rot[k][:],
                                             start=(k == 0), stop=(k == 7))
                        ot = stg.tile([128, VCH], FP32, tag="so", bufs=2,
                                      name=f"ot_n{nn_}m{m}")
                        if m % 2 == 0:
                            nc.vector.tensor_copy(ot[:], ps[:, 0:VCH])
                        else:
                            nc.scalar.activation(out=ot[:], in_=ps[:, 0:VCH],
                                                 func=AF.Copy)
                        nc.sync.dma_start(
                            out_d[ts(m, 128), ts(nn_, VCH)], ot[:])
    nc.compile()
    return nc


def host_prep(inputs):
    idx = np.asarray(inputs["idx"])
    wte = np.asarray(inputs["wte"], np.float32)
    enc = np.asarray(inputs["encoder"], np.float32)
    dx = np.asarray(inputs["decoder_x"], np.float32)
    dy = np.asarray(inputs["decoder_y"], np.float32)
    ro = np.asarray(inputs["readout"], np.float32)
    bf = ml_dtypes.bfloat16

    perm = np.concatenate([np.arange(0, SD, 2), np.arange(1, SD, 2)])
    Wx = np.ascontiguousarray(dx[:, :, perm])                       # [H, D, SD]
    Wy = np.ascontiguousarray(dy[:, :, perm])
    encp = np.ascontiguousarray(enc.reshape(H, SD, D)[:, perm, :])  # [H, SD, D]

    g = wte[idx]                                                    # [B, T, D]
    m = g.mean(-1, keepdims=True)
    var = ((g - m) ** 2).mean(-1, keepdims=True)
    v0 = (g - m) / np.sqrt(var + EPS)

    inv_freq = 1.0 / (10000.0 ** (np.arange(0, SD, 2, dtype=np.float32) / SD))
    freqs = np.arange(T, dtype=np.float32)[None, :] * inv_freq[:, None]
    cosT = np.cos(freqs).astype(np.float32)                         # [SD/2, T]
    sinT = np.sin(freqs).astype(np.float32)

    pp, ff = np.mgrid[0:128, 0:512]
    msk = np.stack([(ff > pp + 128 * q) for q in range(4)]).astype(np.float32)

    in_maps = []
    for c in range(NCORES):
        b, hs = c // 4, c % 4
        hsl = slice(4 * hs, 4 * hs + 4)
        v0b = np.ascontiguousarray(v0[b])                           # [T, D]
        v0t = np.ascontiguousarray(v0b.T)                           # [D, T]
        # v0n[a, p, k, dd] = v0[128k+p, 128a+dd]
        v0n = np.ascontiguousarray(
            v0b.reshape(8, 128, 8, 128).transpose(2, 1, 0, 3))
        in_maps.append({
            "v0t": v0t.astype(bf),
            "v0n": v0n.astype(bf),
            "wx": Wx[hsl].astype(bf),
            "wy": Wy[hsl].astype(bf),
            "enc": np.ascontiguousarray(encp[hsl].reshape(NHC * SD, D)).astype(bf),
            "ro": np.ascontiguousarray(ro[:, VSH * hs: VSH * (hs + 1)]).astype(bf),
            "cos": cosT.astype(bf),
            "sin": sinT.astype(bf),
            "msk": msk.astype(bf),
        })
    return in_maps


def make_runner(nc, n_cores=NCORES):
    import jax
    from jax.sharding import Mesh, PartitionSpec
    from jax.experimental.shard_map import shard_map

    bass2jax.install_neuronx_cc_hook()
    partition_name = nc.partition_id_tensor.name if nc.partition_id_tensor else None
    in_names, out_names, out_avals, zero_shapes = [], [], [], []
    for alloc in nc.m.functions[0].allocations:
        if not isinstance(alloc, mybir.MemoryLocationSet):
            continue
        name = alloc.memorylocations[0].name
        if alloc.kind == "ExternalInput":
            if name != partition_name:
                in_names.append(name)
        elif alloc.kind == "ExternalOutput":
            shape = tuple(alloc.tensor_shape)
            dtype = mybir.dt.np(alloc.dtype)
            out_names.append(name)
            out_avals.append(jax.core.ShapedArray(shape, dtype))
            zero_shapes.append((shape, dtype))
    n_params, n_outs = len(in_names), len(out_avals)
    all_in = list(in_names) + list(out_names)
    if partition_name is not None:
        all_in.append(partition_name)

    def _body(*args):
        operands = list(args)
        if partition_name is not None:
            operands.append(bass2jax.partition_id_tensor())
        return tuple(bass2jax._bass_exec_p.bind(
            *operands, out_avals=tuple(out_avals), in_names=tuple(all_in),
            out_names=tuple(out_names), lowering_input_output_aliases=(),
            sim_require_finite=True, sim_require_nnan=True, nc=nc))

    devices = jax.devices()[:n_cores]
    mesh = Mesh(np.asarray(devices), ("core",))
    f = jax.jit(
        shard_map(_body, mesh=mesh,
                  in_specs=(PartitionSpec("core"),) * (n_params + n_outs),
                  out_specs=(PartitionSpec("core"),) * n_outs, check_rep=False),
        keep_unused=True)

    def prep(in_maps):
        concat = [np.concatenate([np.asarray(in_maps[c][k])
                                  for c in range(n_cores)], axis=0)
                  for k in in_names]
        zeros = [np.zeros((n_cores * s[0], *s[1:]), dt) for (s, dt) in zero_shapes]
        return [jax.device_put(x) for x in concat + zeros]

    def run(dev_args):
        outs = f(*dev_args)
        jax.block_until_ready(outs)
        return outs

    def split(outs):
        return [{name: np.asarray(outs[i]).reshape(n_cores, *out_avals[i].shape)[c]
                 for i, name in enumerate(out_names)} for c in range(n_cores)]

    return run, prep, split


def kernel(**inputs) -> np.ndarray:
    if "prog" not in _CACHE:
        nc = build_program()
        _CACHE["prog"] = nc
        _CACHE["runner"] = make_runner(nc)
    run, prep, split = _CACHE["runner"]
    in_maps = host_prep(inputs)
    args = prep(in_maps)
    res = split(run(args))
    out = np.zeros((B, T, VOCAB), np.float32)
    for c in range(NCORES):
        b, hs = c // 4, c % 4
        out[b, :, VSH * hs: VSH * (hs + 1)] = res[c]["logits"]
    return out


# revision 5
# speedup vs baseline: 1.0946x; 1.0946x over previous
"""Trainium2 Bass kernel for nn_BDH_GPU (sparse linear-attention decoder).

Self-contained: builds an SPMD Bass/Tile program for 8 NeuronCores,
shards batch(2) x head-groups(4), runs via PJRT (axon), gathers output.

Sharding: core c -> batch b=c//4, heads [4*(c%4), 4*(c%4)+4).
Per-layer AllReduce (split into two T-halves) within each 4-core group.

v3 structure:
- t-chunks of 512; activation/state tiles split by T-half.
- LN(a) scale folded past the z-matmul and relu: y = relu(z_raw)*x*rs.
- Next layer's x-matmuls for T-half 0 are emitted between the two tail
  halves, so they fill the AR(1)+tail(1) window on the in-order PE queue.
- enc tiles reuse a 16-slot pool so the half-1 chains are held back and
  execute during the AR(half 0) window.
- v transposes on the PE (tensor-engine transpose) during the tail
  windows instead of 64 slow DMA-transpose descriptors on the Sync queue.
- reciprocal_approx_fast instead of vector.reciprocal (~5x).
- wb (AllReduce result) loads issue from the Scalar HWDGE queue so their
  semaphore wait does not block the Sync queue.
"""
import numpy as np
import ml_dtypes

import concourse.bass as bass
import concourse.tile as tile
import concourse.mybir as mybir
from concourse import bacc, bass2jax
from concourse.masks import make_identity

AF = mybir.ActivationFunctionType
FP32 = mybir.dt.float32
BF16 = mybir.dt.bfloat16
ts = bass.ts

D, H, N, VOCAB, L, SD, B, T = 1024, 16, 8192, 32000, 4, 512, 2, 1024
NCORES = 8
NHC = 4           # heads per core
VSH = VOCAB // 4  # vocab shard per core (within batch group) = 8000
VCH = 500         # vocab N-chunk (<=512 f32 psum bank)
NVC = VSH // VCH  # 16
EPS = 1e-5

_CACHE = {}


def build_program(nlayers=L, repeat=1, do_readout=True, collective=True):
    nc = bacc.Bacc("TRN2", target_bir_lowering=False, debug=False,
                   num_devices=NCORES)
    CDT = BF16

    v0t_d = nc.dram_tensor("v0t", [D, T], CDT, kind="ExternalInput")
    v0n_d = nc.dram_tensor("v0n", [8, 128, 8, 128], CDT, kind="ExternalInput")
    wx_d = nc.dram_tensor("wx", [NHC, D, SD], CDT, kind="ExternalInput")
    wy_d = nc.dram_tensor("wy", [NHC, D, SD], CDT, kind="ExternalInput")
    enc_d = nc.dram_tensor("enc", [NHC * SD, D], CDT, kind="ExternalInput")
    ro_d = nc.dram_tensor("ro", [D, VSH], CDT, kind="ExternalInput")
    cos_d = nc.dram_tensor("cos", [SD // 2, T], CDT, kind="ExternalInput")
    sin_d = nc.dram_tensor("sin", [SD // 2, T], CDT, kind="ExternalInput")
    msk_d = nc.dram_tensor("msk", [4, 128, 512], CDT, kind="ExternalInput")
    out_d = nc.dram_tensor("logits", [T, VSH], FP32, kind="ExternalOutput")

    with tile.TileContext(nc) as tc:
        with (
            tc.tile_pool(name="res", bufs=1) as res,
            tc.tile_pool(name="act", bufs=1) as act,
            tc.tile_pool(name="wst", bufs=10) as wst,
            tc.tile_pool(name="sml", bufs=2) as sml,
            tc.tile_pool(name="stg", bufs=2) as stg,
            tc.tile_pool(name="psp", bufs=2, space="PSUM") as psp,
            tc.tile_pool(name="dram", bufs=2, space="DRAM") as dram,
        ):
            def PPROJ(nm):
                return psp.tile([128, 512], FP32, tag="proj", bufs=2, name=nm)

            def PSC(nm):
                return psp.tile([128, 512], FP32, tag="scp", bufs=2, name=nm)

            def PAV(nm):
                return psp.tile([128, 512], FP32, tag="ap", bufs=2, name=nm)

            def PST(nm):
                return psp.tile([128, 512], FP32, tag="stp", bufs=1, name=nm)

            def PEN(nm):
                return psp.tile([128, 512], FP32, tag="encp", bufs=1, name=nm)

            def PTP(nm):
                return psp.tile([128, 128], CDT, tag="scp", bufs=2, name=nm)

            # ---- constants ----
            cosv, sinv, masks = [], [], []
            for i in range(2):
                ct = res.tile([128, T], CDT, name=f"cos{i}")
                nc.sync.dma_start(ct[:], cos_d[ts(i, 128), :])
                cosv.append(ct)
                st_ = res.tile([128, T], CDT, name=f"sin{i}")
                nc.sync.dma_start(st_[:], sin_d[ts(i, 128), :])
                sinv.append(st_)
            for q in range(4):
                mt = res.tile([128, 512], CDT, name=f"msk{q}")
                nc.sync.dma_start(mt[:], msk_d[q])
                masks.append(mt)
            ones = res.tile([128, 128], CDT, name="ones")
            nc.vector.memset(ones[:], 1.0)
            epst = res.tile([128, 1], FP32, name="epst")
            nc.vector.memset(epst[:], EPS)
            ident = res.tile([128, 128], CDT, name="ident")
            make_identity(nc, ident[:])

            # ---- persistent v state: per-(chunk, T-half) tiles ----
            vT = [[res.tile([128, 512], CDT, tag=f"vt{k}_{jj}",
                            name=f"vt{k}_{jj}") for jj in range(2)]
                  for k in range(8)]
            vn = [[res.tile([128, 4, 128], CDT, tag=f"vn{a}_{hf}",
                            name=f"vn{a}_{hf}") for hf in range(2)]
                  for a in range(8)]

            def load_v(sfx):
                for k in range(8):
                    for jj in range(2):
                        nc.sync.dma_start(vT[k][jj][:],
                                          v0t_d[ts(k, 128), ts(jj, 512)])
                for a in range(8):
                    for hf in range(2):
                        nc.sync.dma_start(vn[a][hf][:],
                                          v0n_d[a, :, 4 * hf:4 * hf + 4, :])

            def emit_x_half(tgx, jj):
                """x' = relu(Wx^T @ v) for one T-half, all heads.
                Emitted right after tail(half jj) so the matmuls fill the
                AR/tail window of the other half on the in-order PE queue."""
                xph = {}
                for h in range(NHC):
                    wxt = []
                    for k in range(8):
                        w = wst.tile([128, SD], CDT, tag="wtile",
                                     name=f"wxp_{tgx}h{h}k{k}")
                        nc.sync.dma_start(w[:], wx_d[h, ts(k, 128), :])
                        wxt.append(w)
                    tiles = []
                    for m in range(4):
                        ps = PPROJ(f"xps_{tgx}h{h}m{m}")
                        for k in range(8):
                            nc.tensor.matmul(
                                ps[:], wxt[k][:, ts(m, 128)], vT[k][jj][:],
                                start=(k == 0), stop=(k == 7))
                        t = act.tile([128, 512], CDT, tag=f"xp{m}_{jj}",
                                     bufs=4 if jj == 0 else 2,
                                     name=f"xp{m}_{jj}_{tgx}h{h}")
                        nc.scalar.activation(out=t[:], in_=ps[:], func=AF.Relu)
                        tiles.append(t)
                    xph[h] = tiles
                return xph

            load_v("init")

            for rep in range(repeat):
                if rep > 0:
                    load_v(f"r{rep}")
                xp0 = emit_x_half(f"r{rep}pre", 0)

                for layer in range(nlayers):
                    tg = f"r{rep}l{layer}"
                    ytiles = {}
                    for h in range(NHC):
                        hg = f"{tg}h{h}"
                        # ---- x' T-half 1 (T-half 0 was precomputed) ----
                        wxt = []
                        for k in range(8):
                            w = wst.tile([128, SD], CDT, tag="wtile",
                                         name=f"wx_{hg}k{k}")
                            nc.sync.dma_start(w[:], wx_d[h, ts(k, 128), :])
                            wxt.append(w)
                        wyt = []
                        for k in range(8):
                            w = wst.tile([128, SD], CDT, tag="wytile", bufs=9,
                                         name=f"wy_{hg}k{k}")
                            nc.sync.dma_start(w[:], wy_d[h, ts(k, 128), :])
                            wyt.append(w)
                        xp = [[xp0[h][m], None] for m in range(4)]
                        for m in range(4):
                            ps = PPROJ(f"xps_{hg}m{m}")
                            for k in range(8):
                                nc.tensor.matmul(
                                    ps[:], wxt[k][:, ts(m, 128)],
                                    vT[k][1][:],
                                    start=(k == 0), stop=(k == 7))
                            t = act.tile([128, 512], CDT, tag=f"xp{m}_1",
                                         bufs=2, name=f"xp{m}_1_{hg}")
                            nc.scalar.activation(out=t[:], in_=ps[:],
                                                 func=AF.Relu)
                            xp[m][1] = t
                        # ---- rope (per T-half) -> qr[i][jj] ----
                        qr = [[act.tile([128, 512], CDT, tag=f"qr{i}_{jj}",
                                        bufs=1, name=f"qr{i}_{jj}_{hg}")
                               for jj in range(2)] for i in range(4)]
                        for jj in range(2):
                            tj = ts(jj, 512)
                            for i in range(2):
                                t1 = sml.tile([128, 512], CDT, tag="rt",
                                              bufs=3, name=f"t1_{hg}i{i}j{jj}")
                                t2 = sml.tile([128, 512], CDT, tag="rt",
                                              bufs=3, name=f"t2_{hg}i{i}j{jj}")
                                nc.vector.tensor_mul(t1[:], xp[i][jj][:],
                                                     cosv[i][:, tj])
                                nc.vector.tensor_mul(t2[:], xp[2 + i][jj][:],
                                                     sinv[i][:, tj])
                                nc.vector.tensor_sub(qr[i][jj][:], t1[:], t2[:])
                                t3 = sml.tile([128, 512], CDT, tag="rt",
                                              bufs=3, name=f"t3_{hg}i{i}j{jj}")
                                t4 = sml.tile([128, 512], CDT, tag="rt",
                                              bufs=3, name=f"t4_{hg}i{i}j{jj}")
                                nc.vector.tensor_mul(t3[:], xp[i][jj][:],
                                                     sinv[i][:, tj])
                                nc.vector.tensor_mul(t4[:], xp[2 + i][jj][:],
                                                     cosv[i][:, tj])
                                nc.vector.tensor_add(qr[2 + i][jj][:],
                                                     t3[:], t4[:])

                        # ---- scores (strict lower-tri), t-chunks of 512 ----
                        sct = {}
                        for i in range(8):
                            for j2 in range(i // 4, 2):
                                ps = PSC(f"scp_{hg}i{i}j{j2}")
                                for k in range(4):
                                    nc.tensor.matmul(
                                        ps[:], qr[k][i // 4][:, ts(i % 4, 128)],
                                        qr[k][j2][:],
                                        start=(k == 0), stop=(k == 3))
                                sc = sml.tile([128, 512], CDT, tag="sc",
                                              bufs=11, name=f"sc_{hg}i{i}j{j2}")
                                if i // 4 == j2:
                                    nc.vector.tensor_mul(sc[:], ps[:],
                                                         masks[i % 4][:])
                                else:
                                    nc.scalar.activation(out=sc[:], in_=ps[:],
                                                         func=AF.Copy)
                                sct[(i, j2)] = sc

                        # per T-half: attnV + stats, then z and y
                        yt = [[act.tile([128, 512], CDT, tag=f"y{h}_{m}_{jj}",
                                        name=f"y{h}_{m}_{jj}_{tg}")
                               for jj in range(2)] for m in range(4)]
                        for j2 in range(2):
                            ns = 4 * (j2 + 1)
                            af = [act.tile([128, 512], CDT, tag="af", bufs=9,
                                           name=f"af{d8}_{j2}_{hg}")
                                  for d8 in range(8)]
                            stp = PST(f"stp_{hg}j{j2}")
                            for d8 in range(8):
                                ps = PAV(f"ap_{hg}j{j2}d{d8}")
                                for i in range(ns):
                                    nc.tensor.matmul(
                                        ps[:], vn[d8][i // 4][:, i % 4, :],
                                        sct[(i, j2)][:],
                                        start=(i == 0), stop=(i == ns - 1))
                                nc.scalar.activation(out=af[d8][:],
                                                     in_=ps[:], func=AF.Copy)
                                sq = sml.tile([128, 512], CDT, tag="sq",
                                              bufs=2, name=f"sq_{hg}j{j2}d{d8}")
                                nc.vector.tensor_mul(sq[:], af[d8][:],
                                                     af[d8][:])
                                nc.tensor.matmul(stp[:], ones[:], sq[:],
                                                 start=(d8 == 0),
                                                 stop=(d8 == 7))
                            sd = sml.tile([128, 512], FP32, tag="sd", bufs=1,
                                          name=f"sd_{hg}j{j2}")
                            nc.scalar.activation(out=sd[:], in_=stp[:],
                                                 func=AF.Sqrt, bias=epst[:],
                                                 scale=1.0 / D)
                            rs = sml.tile([128, 512], FP32, tag=f"rs{j2}",
                                          bufs=1, name=f"rs_{hg}j{j2}")
                            nc.vector.reciprocal_approx_fast(rs[:], sd[:])
                            # ---- z = Wy^T @ a ; y = relu(z)*x*rs ----
                            for m in range(4):
                                ps = PPROJ(f"zps_{hg}m{m}j{j2}")
                                for k in range(8):
                                    nc.tensor.matmul(
                                        ps[:], wyt[k][:, ts(m, 128)],
                                        af[k][:],
                                        start=(k == 0), stop=(k == 7))
                                rl = sml.tile([128, 512], CDT, tag="rl",
                                              bufs=2, name=f"rl_{hg}m{m}j{j2}")
                                nc.scalar.activation(out=rl[:], in_=ps[:],
                                                     func=AF.Relu)
                                nc.vector.tensor_mul(rl[:], rl[:],
                                                     xp[m][j2][:])
                                nc.vector.tensor_mul(yt[m][j2][:], rl[:],
                                                     rs[:])
                        ytiles[h] = yt

                    # ---- w partial = sum_h enc_h^T @ y_h (per T-half) ----
                    ar_in, ar_out = [], []
                    for jj in range(2):
                        ai = dram.tile([D, 512], CDT, tag=f"ar_in{jj}",
                                       name=f"ari{jj}_{tg}")
                        ao = dram.tile([D, 512], CDT, tag=f"ar_out{jj}",
                                       name=f"aro{jj}_{tg}")
                        ar_in.append(ai)
                        ar_out.append(ao)
                    for jj in range(2):
                        # etile bufs=16: each (jj,dq) group reuses the prior
                        # group's slots, holding back the half-1 chains so
                        # they execute during the AR(half 0) window.
                        for dq in range(2):
                            ech = []
                            for kk in range(16):
                                e = wst.tile([128, 512], CDT, tag="etile",
                                             bufs=16,
                                             name=f"e_{tg}j{jj}q{dq}k{kk}")
                                nc.sync.dma_start(
                                    e[:], enc_d[ts(kk, 128), ts(dq, 512)])
                                ech.append(e)
                            for dl in range(4):
                                d8 = 4 * dq + dl
                                ps = PEN(f"ep_{tg}j{jj}d{d8}")
                                for kk in range(16):
                                    hh, mm = kk // 4, kk % 4
                                    nc.tensor.matmul(
                                        ps[:], ech[kk][:, ts(dl, 128)],
                                        ytiles[hh][mm][jj][:],
                                        start=(kk == 0), stop=(kk == 15))
                                so = stg.tile([128, 512], CDT, tag="so",
                                              bufs=2, name=f"so_{tg}j{jj}d{d8}")
                                nc.vector.tensor_copy(so[:], ps[:])
                                nc.sync.dma_start(ar_in[jj][ts(d8, 128), :],
                                                  so[:])
                        if collective:
                            nc.gpsimd.collective_compute(
                                "AllReduce", mybir.AluOpType.add,
                                replica_groups=[[0, 1, 2, 3], [4, 5, 6, 7]],
                                ins=[ar_in[jj].opt()], outs=[ar_out[jj].opt()])
                    w_srcs = ar_out if collective else ar_in

                    # ---- tail: u=LN(w); s=v+u; vnew=s*rsqrt(msq(s)+eps) ----
                    for jj in range(2):
                        wb = []
                        mwp = PEN(f"mwp_{tg}j{jj}")
                        msp = PST(f"msp_{tg}j{jj}")
                        for k in range(8):
                            wbk = sml.tile([128, 512], CDT, tag=f"wb{k}",
                                           bufs=1, name=f"wb_{tg}j{jj}k{k}")
                            nc.scalar.dma_start(wbk[:],
                                                w_srcs[jj][ts(k, 128), :])
                            wb.append(wbk)
                            sqw = sml.tile([128, 512], CDT, tag="sqw", bufs=2,
                                           name=f"sqw_{tg}j{jj}k{k}")
                            nc.scalar.activation(out=sqw[:], in_=wbk[:],
                                                 func=AF.Square)
                            nc.tensor.matmul(mwp[:], ones[:], wbk[:],
                                             start=(k == 0), stop=(k == 7))
                            nc.tensor.matmul(msp[:], ones[:], sqw[:],
                                             start=(k == 0), stop=(k == 7))
                        mwn = sml.tile([128, 512], FP32, tag="tf32", bufs=3,
                                       name=f"mwn_{tg}j{jj}")
                        nc.scalar.activation(out=mwn[:], in_=mwp[:],
                                             func=AF.Copy, scale=1.0 / D)
                        vt = sml.tile([128, 512], FP32, tag="tf32", bufs=3,
                                      name=f"vt_{tg}j{jj}")
                        nc.scalar.activation(out=vt[:], in_=msp[:],
                                             func=AF.Copy, scale=1.0 / D)
                        m2 = sml.tile([128, 512], FP32, tag="tf32", bufs=3,
                                      name=f"m2_{tg}j{jj}")
                        nc.vector.tensor_mul(m2[:], mwn[:], mwn[:])
                        nc.vector.tensor_sub(vt[:], vt[:], m2[:])
                        nc.scalar.activation(out=vt[:], in_=vt[:],
                                             func=AF.Sqrt, bias=epst[:],
                                             scale=1.0)
                        rsw = sml.tile([128, 512], FP32, tag="tf32", bufs=3,
                                       name=f"rsw_{tg}j{jj}")
                        nc.vector.reciprocal_approx_fast(rsw[:], vt[:])
                        st = [sml.tile([128, 512], CDT, tag=f"st{k}", bufs=1,
                                       name=f"st{k}_{tg}j{jj}")
                              for k in range(8)]
                        ssp = PEN(f"ssp_{tg}j{jj}")
                        for k in range(8):
                            u = sml.tile([128, 512], CDT, tag="tu", bufs=2,
                                         name=f"u_{tg}j{jj}k{k}")
                            nc.vector.tensor_sub(u[:], wb[k][:], mwn[:])
                            nc.vector.tensor_mul(u[:], u[:], rsw[:])
                            nc.gpsimd.tensor_add(st[k][:], vT[k][jj][:], u[:])
                            sq2 = sml.tile([128, 512], CDT, tag="sqw", bufs=2,
                                           name=f"sq2_{tg}j{jj}k{k}")
                            nc.scalar.activation(out=sq2[:], in_=st[k][:],
                                                 func=AF.Square)
                            nc.tensor.matmul(ssp[:], ones[:], sq2[:],
                                             start=(k == 0), stop=(k == 7))
                        sd3 = sml.tile([128, 512], FP32, tag="tf32", bufs=3,
                                       name=f"sd3_{tg}j{jj}")
                        nc.scalar.activation(out=sd3[:], in_=ssp[:],
                                             func=AF.Sqrt, bias=epst[:],
                                             scale=1.0 / D)
                        rs2 = sml.tile([128, 512], FP32, tag="tf32", bufs=3,
                                       name=f"rs2_{tg}j{jj}")
                        nc.vector.reciprocal_approx_fast(rs2[:], sd3[:])
                        for k in range(8):
                            nc.vector.tensor_mul(vT[k][jj][:], st[k][:],
                                                 rs2[:])
                        # transpose vnew half -> natural layout on the PE
                        for a in range(8):
                            for kq in range(4):
                                tp = PTP(f"tp_{tg}j{jj}a{a}q{kq}")
                                nc.tensor.transpose(
                                    tp[:], vT[a][jj][:, ts(kq, 128)],
                                    ident[:])
                                nc.vector.tensor_copy(vn[a][jj][:, kq, :],
                                                      tp[:])
                        if jj == 0 and layer + 1 < nlayers:
                            xp0 = emit_x_half(f"{tg}pre", 0)

            # ---- readout: logits = v^T @ readout_shard ----
            if do_readout:
                for nn_ in range(NVC):
                    rot = []
                    for k in range(8):
                        rtile = wst.tile([128, VCH], CDT, tag="wtile", bufs=10,
                                         name=f"ro_n{nn_}k{k}")
                        nc.sync.dma_start(
                            rtile[:], ro_d[ts(k, 128), ts(nn_, VCH)])
                        rot.append(rtile)
                    for m in range(8):
                        ps = PPROJ(f"rps_n{nn_}m{m}")
                        for k in range(8):
                            nc.tensor.matmul(ps[:, 0:VCH],
                                             vT[k][m // 4][:, ts(m % 4, 128)],
                                             rot[k][:],
                                             start=(k == 0), stop=(k == 7))
                        ot = stg.tile([128, VCH], FP32, tag="so", bufs=2,
                                      name=f"ot_n{nn_}m{m}")
                        if m % 2 == 0:
                            nc.vector.tensor_copy(ot[:], ps[:, 0:VCH])
                        else:
                            nc.scalar.activation(out=ot[:], in_=ps[:, 0:VCH],
                                                 func=AF.Copy)
                        nc.sync.dma_start(
                            out_d[ts(m, 128), ts(nn_, VCH)], ot[:])
    nc.compile()
    return nc


def host_prep(inputs):
    idx = np.asarray(inputs["idx"])
    wte = np.asarray(inputs["wte"], np.float32)
    enc = np.asarray(inputs["encoder"], np.float32)
    dx = np.asarray(inputs["decoder_x"], np.float32)
    dy = np.asarray(inputs["decoder_y"], np.float32)
    ro = np.asarray(inputs["readout"], np.float32)
    bf = ml_dtypes.bfloat16

    perm = np.concatenate([np.arange(0, SD, 2), np.arange(1, SD, 2)])
    Wx = np.ascontiguousarray(dx[:, :, perm])                       # [H, D, SD]
    Wy = np.ascontiguousarray(dy[:, :, perm])
    encp = np.ascontiguousarray(enc.reshape(H, SD, D)[:, perm, :])  # [H, SD, D]

    g = wte[idx]                                                    # [B, T, D]
    m = g.mean(-1, keepdims=True)
    var = ((g - m) ** 2).mean(-1, keepdims=True)
    v0 = (g - m) / np.sqrt(var + EPS)

    inv_freq = 1.0 / (10000.0 ** (np.arange(0, SD, 2, dtype=np.float32) / SD))
    freqs = np.arange(T, dtype=np.float32)[None, :] * inv_freq[:, None]
    cosT = np.cos(freqs).astype(np.float32)                         # [SD/2, T]
    sinT = np.sin(freqs).astype(np.float32)

    pp, ff = np.mgrid[0:128, 0:512]
    msk = np.stack([(ff > pp + 128 * q) for q in range(4)]).astype(np.float32)

    in_maps = []
    for c in range(NCORES):
        b, hs = c // 4, c % 4
        hsl = slice(4 * hs, 4 * hs + 4)
        v0b = np.ascontiguousarray(v0[b])                           # [T, D]
        v0t = np.ascontiguousarray(v0b.T)                           # [D, T]
        # v0n[a, p, k, dd] = v0[128k+p, 128a+dd]
        v0n = np.ascontiguousarray(
            v0b.reshape(8, 128, 8, 128).transpose(2, 1, 0, 3))
        in_maps.append({
            "v0t": v0t.astype(bf),
            "v0n": v0n.astype(bf),
            "wx": Wx[hsl].astype(bf),
            "wy": Wy[hsl].astype(bf),
            "enc": np.ascontiguousarray(encp[hsl].reshape(NHC * SD, D)).astype(bf),
            "ro": np.ascontiguousarray(ro[:, VSH * hs: VSH * (hs + 1)]).astype(bf),
            "cos": cosT.astype(bf),
            "sin": sinT.astype(bf),
            "msk": msk.astype(bf),
        })
    return in_maps


def make_runner(nc, n_cores=NCORES):
    import jax
    from jax.sharding import Mesh, PartitionSpec
    from jax.experimental.shard_map import shard_map

    bass2jax.install_neuronx_cc_hook()
    partition_name = nc.partition_id_tensor.name if nc.partition_id_tensor else None
    in_names, out_names, out_avals, zero_shapes = [], [], [], []
    for alloc in nc.m.functions[0].allocations:
        if not isinstance(alloc, mybir.MemoryLocationSet):
            continue
        name = alloc.memorylocations[0].name
        if alloc.kind == "ExternalInput":
            if name != partition_name:
                in_names.append(name)
        elif alloc.kind == "ExternalOutput":
            shape = tuple(alloc.tensor_shape)
            dtype = mybir.dt.np(alloc.dtype)
            out_names.append(name)
            out_avals.append(jax.core.ShapedArray(shape, dtype))
            zero_shapes.append((shape, dtype))
    n_params, n_outs = len(in_names), len(out_avals)
    all_in = list(in_names) + list(out_names)
    if partition_name is not None:
        all_in.append(partition_name)

    def _body(*args):
        operands = list(args)
        if partition_name is not None:
            operands.append(bass2jax.partition_id_tensor())
        return tuple(bass2jax._bass_exec_p.bind(
            *operands, out_avals=tuple(out_avals), in_names=tuple(all_in),
            out_names=tuple(out_names), lowering_input_output_aliases=(),
            sim_require_finite=True, sim_require_nnan=True, nc=nc))

    devices = jax.devices()[:n_cores]
    mesh = Mesh(np.asarray(devices), ("core",))
    f = jax.jit(
        shard_map(_body, mesh=mesh,
                  in_specs=(PartitionSpec("core"),) * (n_params + n_outs),
                  out_specs=(PartitionSpec("core"),) * n_outs, check_rep=False),
        keep_unused=True)

    def prep(in_maps):
        concat = [np.concatenate([np.asarray(in_maps[c][k])
                                  for c in range(n_cores)], axis=0)
                  for k in in_names]
        zeros = [np.zeros((n_cores * s[0], *s[1:]), dt) for (s, dt) in zero_shapes]
        return [jax.device_put(x) for x in concat + zeros]

    def run(dev_args):
        outs = f(*dev_args)
        jax.block_until_ready(outs)
        return outs

    def split(outs):
        return [{name: np.asarray(outs[i]).reshape(n_cores, *out_avals[i].shape)[c]
                 for i, name in enumerate(out_names)} for c in range(n_cores)]

    return run, prep, split


def kernel(**inputs) -> np.ndarray:
    if "prog" not in _CACHE:
        nc = build_program()
        _CACHE["prog"] = nc
        _CACHE["runner"] = make_runner(nc)
    run, prep, split = _CACHE["runner"]
    in_maps = host_prep(inputs)
    args = prep(in_maps)
    res = split(run(args))
    out = np.zeros((B, T, VOCAB), np.float32)
    for c in range(NCORES):
        b, hs = c // 4, c % 4
        out[b, :, VSH * hs: VSH * (hs + 1)] = res[c]["logits"]
    return out
